# revision 6
# baseline (speedup 1.0000x reference)
"""Trainium2 Bass kernel: causal multi-head attention block (B=2, T=2048, C=1024, H=16).

Sharding: 8 cores = 2 (batch) x 4 (head groups of 4 heads).  Each core computes
q/k/v projections for its 4 heads, causal attention, and a partial out-proj
(rows of wo for its head slice).  Host sums the 4 partials per batch element
and adds bo.

v2 restructure (vs baseline):
  - Strip-pipelined: for each 512-row strip st, emit out-proj(st-1),
    projections(st), attention(st).  Causality means attention strip st only
    needs k/v prefix data, so ACT-exp work starts ~12us in instead of ~45us,
    and the PE never idles long enough for HAM to re-throttle to 1.2 GHz.
  - q/k stored per head-pair [128, T] with head h at partitions 64*(h%2)..;
    score matmuls contract 64 partitions at matching base partition.  No
    zero-padding, no big startup memsets.
  - exp batched 2 heads per ACT instruction (sAB [128, 1024] across 2 PSUM
    banks) -> ~80 instead of 160 activations.
  - Softmax denominator via ones-column in v (PSUM row 64); reciprocal +
    SBUF->SBUF broadcast DMA + fused normalize, all on-chip (no DRAM bounce).
  - y stored bf16 (halves DMA-out), bias bo + partial sum on host.
  - Scalar engine does ONLY exp; DMA triggers live on sync/gpsimd.
"""

import os
import sys

import numpy as np

try:
    import ml_dtypes
    BF16_NP = ml_dtypes.bfloat16
except ImportError:  # pragma: no cover
    BF16_NP = None

for _p in ("/opt/trn_rl_repo", "/root/.axon_site/_ro/trn_rl_repo"):
    if os.path.isdir(_p) and _p not in sys.path:
        sys.path.append(_p)

import concourse.bass as bass  # noqa: E402
import concourse.mybir as mybir  # noqa: E402
import concourse.tile as tile  # noqa: E402

F32 = mybir.dt.float32
BF16 = mybir.dt.bfloat16

B, T, C, H = 2, 2048, 1024, 16
D = C // H          # 64
HPC = 4             # heads per core
DPC = HPC * D       # 256 head-dims per core
NCORES = 8

CHUNK = 128         # contraction / s-chunk granularity
STRIP = 512         # t-strip width
VW = D + 1          # 65: v columns + ones column per head

_CTRL_TYPES = (mybir.InstDrain, mybir.InstNoOp, mybir.InstEventSemaphore)


def split_excess_waits(nc, lim=1):
    """Walrus in this toolchain accepts at most one sync-wait per instruction.
    Move extra waits onto same-engine NoOps inserted just before the owner."""
    k = 0
    for fn in nc.m.functions:
        for blk in fn.blocks:
            out = []
            changed = False
            for inst in blk.instructions:
                si = inst.sync_info
                if si is not None and si.on_wait and len(si.on_wait) > lim:
                    waits = list(si.on_wait)
                    extra, keep = waits[:-lim], waits[-lim:]
                    for w in extra:
                        nop = mybir.InstNoOp(name=f"waitfix_{k}", ins=[], outs=[])
                        k += 1
                        nop.engine = inst.engine
                        nop.sync_info = mybir.SyncInfo(on_wait=[w], on_update=[])
                        out.append(nop)
                    si.on_wait = keep
                    changed = True
                out.append(inst)
            if changed:
                blk.instructions = out
    return k


def build_nc(t_len=T, fix_waits=True):
    """Build the per-core SPMD Bass program (same program on all 8 cores)."""
    assert t_len % STRIP == 0
    n_strip = t_len // STRIP          # 4 at full size
    n_ttile = t_len // CHUNK          # 16 at full size
    n_cchunk = C // CHUNK             # 8
    half_len = min(t_len, 1024)

    nc = bass.Bass(target_bir_lowering=False)

    xT = nc.dram_tensor("xT", [C, t_len], BF16, kind="ExternalInput")
    wqkvT = nc.dram_tensor("wqkvT", [C, 3 * DPC], BF16, kind="ExternalInput")
    woT = nc.dram_tensor("woT", [DPC, C], BF16, kind="ExternalInput")
    bqk = nc.dram_tensor("bqk", [CHUNK, 4], F32, kind="ExternalInput")
    bv_row = nc.dram_tensor("bv_row", [1, DPC], F32, kind="ExternalInput")
    y = nc.dram_tensor("y", [t_len, C], BF16, kind="ExternalOutput")

    Exp = mybir.ActivationFunctionType.Exp

    with tile.TileContext(nc) as tc:
        with tc.tile_pool(name="persist", bufs=1) as pp:
            # ---- input DMAs first (sync engine), so PE can start ASAP ----
            w_sb = []
            for c in range(n_cchunk):
                w = pp.tile([CHUNK, 3 * DPC], BF16, tag=f"w{c}")
                nc.sync.dma_start(out=w, in_=wqkvT[c * CHUNK:(c + 1) * CHUNK, :])
                w_sb.append(w)
            # x tiles [c-chunk][half]: ordered so strip 0's needs come first
            n_half = t_len // half_len
            xt = [[None] * n_half for _ in range(n_cchunk)]
            for half in range(n_half):
                for c in range(n_cchunk):
                    x_ = pp.tile([CHUNK, half_len], BF16, tag=f"x{c}_{half}",
                                 name=f"x{c}_{half}")
                    eng = nc.sync if c % 2 == 0 else nc.gpsimd
                    eng.dma_start(
                        out=x_,
                        in_=xT[c * CHUNK:(c + 1) * CHUNK,
                               half * half_len:(half + 1) * half_len])
                    xt[c][half] = x_
            wo_sb = []
            for i in range(2):
                w = pp.tile([CHUNK, C], BF16, tag=f"wo{i}")
                nc.sync.dma_start(out=w, in_=woT[i * CHUNK:(i + 1) * CHUNK, :])
                wo_sb.append(w)
            bqk_sb = pp.tile([CHUNK, 4], F32, tag="bqk")
            nc.gpsimd.dma_start(out=bqk_sb, in_=bqk[:, :])
            bv_bc = pp.tile([CHUNK, DPC], F32, tag="bv_bc")
            nc.gpsimd.dma_start(out=bv_bc, in_=bv_row[0:1, :].broadcast_to((CHUNK, DPC)))

            # ---- constants ----
            trimask = pp.tile([CHUNK, CHUNK], BF16, tag="trimask")
            nc.gpsimd.memset(trimask, 1.0)
            # keep 1.0 where t - s >= 0 (upper triangular incl. diagonal)
            nc.gpsimd.affine_select(
                out=trimask, in_=trimask,
                pattern=[[1, CHUNK]], channel_multiplier=-1, base=0,
                compare_op=mybir.AluOpType.is_ge, fill=0.0,
            )

            # ---- persistent activations ----
            # qT/kT per head-pair: [128, T], head (2m+hf) at partitions 64hf..
            qT = [pp.tile([CHUNK, t_len], BF16, tag=f"qT{m}", name=f"qT{m}")
                  for m in range(2)]
            kT = [pp.tile([CHUNK, t_len], BF16, tag=f"kT{m}", name=f"kT{m}")
                  for m in range(2)]
            # v tiles: [s-chunk 128, 4 heads x (64 v | ones)]
            vaug = [pp.tile([CHUNK, HPC * VW], BF16, tag=f"v{j}", name=f"v{j}")
                    for j in range(n_ttile)]
            for j in range(n_ttile):
                ones_cols = vaug[j].rearrange("p (h e) -> p h e", e=VW)[:, :, D:D + 1]
                nc.vector.memset(ones_cols, 1.0)
            aoT = [pp.tile([CHUNK, t_len], BF16, tag=f"aoT{m}", name=f"aoT{m}")
                   for m in range(2)]

            with tc.tile_pool(name="sb2", bufs=1) as sp, \
                 tc.tile_pool(name="dr", bufs=1, space="DRAM") as dr, \
                 tc.tile_pool(name="ps_s", bufs=1, space="PSUM") as pss, \
                 tc.tile_pool(name="ps_q", bufs=1, space="PSUM") as psq, \
                 tc.tile_pool(name="ps_av", bufs=1, space="PSUM") as psav:

                def emit_ph1(st):
                    """Projections for t-strip st: v into vaug, q/k into qT/kT."""
                    s0 = st * STRIP
                    half, off = s0 // half_len, s0 % half_len
                    for tsub in range(STRIP // CHUNK):
                        jt = st * (STRIP // CHUNK) + tsub
                        pv = psq.tile([CHUNK, STRIP], F32, tag="pq", bufs=2,
                                      name=f"pv{jt}")
                        for c in range(n_cchunk):
                            nc.tensor.matmul(
                                pv[:, 0:DPC],
                                xt[c][half][:, off + tsub * CHUNK:
                                            off + (tsub + 1) * CHUNK],
                                w_sb[c][:, 2 * DPC:3 * DPC],
                                start=(c == 0), stop=(c == n_cchunk - 1))
                        nc.vector.tensor_add(
                            vaug[jt].rearrange("p (h e) -> p h e", e=VW)[:, :, 0:D],
                            pv[:, 0:DPC].rearrange("p (h d) -> p h d", d=D),
                            bv_bc.rearrange("p (h d) -> p h d", d=D))
                    for m in range(2):            # head pair
                        for pj in range(2):       # 0 = q, 1 = k
                            pq = psq.tile([CHUNK, STRIP], F32, tag="pq", bufs=2,
                                          name=f"pq{pj}_{m}_{st}")
                            for c in range(n_cchunk):
                                nc.tensor.matmul(
                                    pq,
                                    w_sb[c][:, pj * DPC + m * CHUNK:
                                            pj * DPC + (m + 1) * CHUNK],
                                    xt[c][half][:, off:off + STRIP],
                                    start=(c == 0), stop=(c == n_cchunk - 1))
                            dst = (qT if pj == 0 else kT)[m][:, s0:s0 + STRIP]
                            nc.vector.tensor_scalar_add(
                                dst, pq, bqk_sb[:, 2 * pj + m:2 * pj + m + 1])

                def emit_ph2(st):
                    """Causal attention for t-strip st, both head pairs."""
                    s0 = st * STRIP
                    nj = 4 * st + 4
                    for pair in range(2):
                        avA = psav.tile([CHUNK, STRIP], F32, tag="avA", bufs=1,
                                        name=f"avA{pair}_{st}")
                        avB = psav.tile([CHUNK, STRIP], F32, tag="avB", bufs=1,
                                        name=f"avB{pair}_{st}")
                        for j in range(nj):
                            t0 = max(s0, CHUNK * j)
                            L = s0 + STRIP - t0
                            off = t0 - s0
                            sAB = pss.tile([CHUNK, 2 * STRIP], F32, tag="sAB",
                                           bufs=2, name=f"s{pair}_{st}_{j}")
                            for hf in range(2):
                                nc.tensor.matmul(
                                    sAB[:, hf * STRIP:hf * STRIP + L],
                                    kT[pair][64 * hf:64 * hf + 64,
                                             j * CHUNK:(j + 1) * CHUNK],
                                    qT[pair][64 * hf:64 * hf + 64, t0:t0 + L],
                                    start=True, stop=True)
                            eAB = sp.tile([CHUNK, 2 * STRIP], BF16, tag="eAB",
                                          bufs=3, name=f"e{pair}_{st}_{j}")
                            if L == STRIP:
                                nc.scalar.activation(eAB, sAB, Exp)
                            else:
                                nc.scalar.activation(
                                    eAB.rearrange("p (h f) -> p h f",
                                                  f=STRIP)[:, :, 0:L],
                                    sAB.rearrange("p (h f) -> p h f",
                                                  f=STRIP)[:, :, 0:L],
                                    Exp)
                            if CHUNK * j >= s0:  # diagonal tile
                                nc.vector.tensor_mul(
                                    eAB[:, 0:CHUNK], eAB[:, 0:CHUNK], trimask)
                                nc.vector.tensor_mul(
                                    eAB[:, STRIP:STRIP + CHUNK],
                                    eAB[:, STRIP:STRIP + CHUNK], trimask)
                            for hf, av in ((0, avA), (1, avB)):
                                h = 2 * pair + hf
                                nc.tensor.matmul(
                                    av[0:VW, off:STRIP],
                                    vaug[j][:, h * VW:(h + 1) * VW],
                                    eAB[:, hf * STRIP:hf * STRIP + L],
                                    start=(j == 0), stop=(j == nj - 1),
                                    skip_group_check=True)
                        # normalize: recip of denom rows, DRAM-bounce broadcast,
                        # fused scale
                        bcf = sp.tile([CHUNK, STRIP], BF16, tag="bcf", bufs=2,
                                      name=f"bcf{pair}_{st}")
                        rrd = dr.tile([2, STRIP], BF16, tag="rrd", bufs=2,
                                      name=f"rrd_{pair}_{st}")
                        for hf, av in ((0, avA), (1, avB)):
                            rr = sp.tile([1, STRIP], BF16, tag=f"rr{hf}",
                                         bufs=2, name=f"rr{hf}_{pair}_{st}")
                            with nc.allow_low_precision(reason="1/denom in bf16"):
                                nc.vector.reciprocal(rr, av[D:D + 1, :])
                            nc.gpsimd.dma_start(out=rrd[hf:hf + 1, :], in_=rr)
                            nc.gpsimd.dma_start(
                                out=bcf[64 * hf:64 * hf + D, :],
                                in_=rrd[hf:hf + 1, :].broadcast_to((D, STRIP)))
                        # head A: partitions align -> fused multiply
                        nc.vector.tensor_mul(
                            aoT[pair][0:D, s0:s0 + STRIP],
                            avA[0:D, :], bcf[0:D, :])
                        # head B: copy (cross-partition, proven) then in-place mul
                        nc.vector.tensor_copy(
                            aoT[pair][64:64 + D, s0:s0 + STRIP], avB[0:D, :])
                        nc.vector.tensor_mul(
                            aoT[pair][64:64 + D, s0:s0 + STRIP],
                            aoT[pair][64:64 + D, s0:s0 + STRIP],
                            bcf[64:64 + D, :])

                def emit_ph3(st):
                    """Out-proj partial for t-strip st; y stored bf16."""
                    for tsub in range(STRIP // CHUNK):
                        jt = st * (STRIP // CHUNK) + tsub
                        ysb = sp.tile([CHUNK, C], BF16, tag="ysb", bufs=3,
                                      name=f"ysb{jt}")
                        for js in range(2):
                            py = psq.tile([CHUNK, STRIP], F32, tag="pq", bufs=2,
                                          name=f"py{jt}_{js}")
                            for pair in range(2):
                                nc.tensor.matmul(
                                    py,
                                    aoT[pair][:, jt * CHUNK:(jt + 1) * CHUNK],
                                    wo_sb[pair][:, js * STRIP:(js + 1) * STRIP],
                                    start=(pair == 0), stop=(pair == 1))
                            nc.vector.tensor_copy(
                                ysb[:, js * STRIP:(js + 1) * STRIP], py)
                        nc.gpsimd.dma_start(
                            out=y[jt * CHUNK:(jt + 1) * CHUNK, :], in_=ysb)

                for st in range(n_strip):
                    if st > 0:
                        emit_ph3(st - 1)
                    emit_ph1(st)
                    emit_ph2(st)
                emit_ph3(n_strip - 1)

    if fix_waits:
        split_excess_waits(nc)
    return nc


def make_in_maps(x, wq, bq, wk, bk, wv, bv, wo, bo=None, t_len=T):
    """Build the 8 per-core input dicts from full inputs (bo handled on host)."""
    in_maps = []
    scale = 1.0 / np.sqrt(np.float32(D))
    for core in range(NCORES):
        b, hg = core // 4, core % 4
        sl = slice(DPC * hg, DPC * (hg + 1))
        wqs = (wq[sl] * scale).astype(np.float32)
        bqs = (bq[sl] * scale).astype(np.float32)
        wqkvT = np.concatenate([wqs.T, wk[sl].T, wv[sl].T], axis=1)
        bqk = np.stack([bqs[0:CHUNK], bqs[CHUNK:2 * CHUNK],
                        bk[sl][0:CHUNK], bk[sl][CHUNK:2 * CHUNK]], axis=1)
        in_maps.append({
            "xT": np.ascontiguousarray(x[b, :t_len].T).astype(BF16_NP),
            "wqkvT": np.ascontiguousarray(wqkvT).astype(BF16_NP),
            "woT": np.ascontiguousarray(wo[:, sl].T).astype(BF16_NP),
            "bqk": np.ascontiguousarray(bqk, dtype=np.float32),
            "bv_row": np.ascontiguousarray(bv[sl][None, :], dtype=np.float32),
        })
    return in_maps


def gather_output(results, bo, t_len=T):
    ys = [np.asarray(results[i]["y"], dtype=np.float32) for i in range(NCORES)]
    out = np.stack([ys[0] + ys[1] + ys[2] + ys[3],
                    ys[4] + ys[5] + ys[6] + ys[7]])
    return (out + np.asarray(bo, dtype=np.float32)[None, None, :]).astype(np.float32)


_NC_CACHE = {}


def _get_nc(t_len=T):
    if t_len not in _NC_CACHE:
        _NC_CACHE[t_len] = build_nc(t_len)
    return _NC_CACHE[t_len]


def kernel(x, wq, bq, wk, bk, wv, bv, wo, bo, mask=None, **_unused):
    """Full-input entry point: shard, run on 8 NeuronCores, gather."""
    from concourse.bass_utils import run_bass_kernel_spmd

    x = np.asarray(x, dtype=np.float32)
    in_maps = make_in_maps(x, np.asarray(wq, np.float32), np.asarray(bq, np.float32),
                           np.asarray(wk, np.float32), np.asarray(bk, np.float32),
                           np.asarray(wv, np.float32), np.asarray(bv, np.float32),
                           np.asarray(wo, np.float32))
    nc = _get_nc(T)
    res = run_bass_kernel_spmd(nc, in_maps, list(range(NCORES)))
    return gather_output(res.results, bo)


# revision 15
# speedup vs baseline: 1.1089x; 1.1089x over previous
"""Trainium2 Bass kernel: causal multi-head attention block (B=2, T=2048, C=1024, H=16).

Sharding: 8 cores = 2 (batch) x 4 (head groups of 4 heads).  Each core computes
q/k/v projections for its 4 heads, causal attention, and a partial out-proj
(rows of wo for its head slice).  Host sums the 4 partials per batch element
and adds bo.

v2 restructure (vs baseline):
  - Strip-pipelined: for each 512-row strip st, emit out-proj(st-1),
    projections(st), attention(st).  Causality means attention strip st only
    needs k/v prefix data, so ACT-exp work starts ~12us in instead of ~45us,
    and the PE never idles long enough for HAM to re-throttle to 1.2 GHz.
  - q/k stored per head-pair [128, T] with head h at partitions 64*(h%2)..;
    score matmuls contract 64 partitions at matching base partition.  No
    zero-padding, no big startup memsets.
  - exp batched 2 heads per ACT instruction (sAB [128, 1024] across 2 PSUM
    banks) -> ~80 instead of 160 activations.
  - Softmax denominator via ones-column in v (PSUM row 64); reciprocal +
    SBUF->SBUF broadcast DMA + fused normalize, all on-chip (no DRAM bounce).
  - y stored bf16 (halves DMA-out), bias bo + partial sum on host.
  - Scalar engine does ONLY exp; DMA triggers live on sync/gpsimd.
"""

import os
import sys

import numpy as np

try:
    import ml_dtypes
    BF16_NP = ml_dtypes.bfloat16
except ImportError:  # pragma: no cover
    BF16_NP = None

for _p in ("/opt/trn_rl_repo", "/root/.axon_site/_ro/trn_rl_repo"):
    if os.path.isdir(_p) and _p not in sys.path:
        sys.path.append(_p)

import concourse.bass as bass  # noqa: E402
import concourse.mybir as mybir  # noqa: E402
import concourse.tile as tile  # noqa: E402

F32 = mybir.dt.float32
BF16 = mybir.dt.bfloat16

B, T, C, H = 2, 2048, 1024, 16
D = C // H          # 64
HPC = 4             # heads per core
DPC = HPC * D       # 256 head-dims per core
NCORES = 8

CHUNK = 128         # contraction / s-chunk granularity
STRIP = 512         # t-strip width
VW = D + 1          # 65: v columns + ones column per head

_CTRL_TYPES = (mybir.InstDrain, mybir.InstNoOp, mybir.InstEventSemaphore)


def split_excess_waits(nc, lim=1):
    """Walrus in this toolchain accepts at most one sync-wait per instruction.
    Move extra waits onto same-engine NoOps inserted just before the owner."""
    k = 0
    for fn in nc.m.functions:
        for blk in fn.blocks:
            out = []
            changed = False
            for inst in blk.instructions:
                si = inst.sync_info
                if si is not None and si.on_wait and len(si.on_wait) > lim:
                    waits = list(si.on_wait)
                    extra, keep = waits[:-lim], waits[-lim:]
                    for w in extra:
                        nop = mybir.InstNoOp(name=f"waitfix_{k}", ins=[], outs=[])
                        k += 1
                        nop.engine = inst.engine
                        nop.sync_info = mybir.SyncInfo(on_wait=[w], on_update=[])
                        out.append(nop)
                    si.on_wait = keep
                    changed = True
                out.append(inst)
            if changed:
                blk.instructions = out
    return k


def build_nc(t_len=T, fix_waits=True):
    """Build the per-core SPMD Bass program (same program on all 8 cores)."""
    assert t_len % STRIP == 0
    n_strip = t_len // STRIP          # 4 at full size
    n_ttile = t_len // CHUNK          # 16 at full size
    n_cchunk = C // CHUNK             # 8
    half_len = min(t_len, 1024)

    nc = bass.Bass(target_bir_lowering=False)

    xT = nc.dram_tensor("xT", [C, t_len], BF16, kind="ExternalInput")
    wqkvT = nc.dram_tensor("wqkvT", [C, 3 * DPC], BF16, kind="ExternalInput")
    woT = nc.dram_tensor("woT", [DPC, C], BF16, kind="ExternalInput")
    bqk = nc.dram_tensor("bqk", [CHUNK, 4], F32, kind="ExternalInput")
    bv_row = nc.dram_tensor("bv_row", [1, DPC], F32, kind="ExternalInput")
    y = nc.dram_tensor("y", [t_len, C], BF16, kind="ExternalOutput")

    Exp = mybir.ActivationFunctionType.Exp

    with tile.TileContext(nc) as tc:
        with tc.tile_pool(name="persist", bufs=1) as pp:
            # ---- input DMAs first, spread over 4 engine queues so strip-0
            # data lands fast and the PE can start ASAP ----
            dma_engs = [nc.sync, nc.gpsimd, nc.scalar]
            n_half = t_len // half_len
            xt = [[None] * n_half for _ in range(n_cchunk)]
            w_sb = [None] * n_cchunk
            for half in range(n_half):
                for c in range(n_cchunk):
                    x_ = pp.tile([CHUNK, half_len], BF16, tag=f"x{c}_{half}",
                                 name=f"x{c}_{half}")
                    dma_engs[c % 3].dma_start(
                        out=x_,
                        in_=xT[c * CHUNK:(c + 1) * CHUNK,
                               half * half_len:(half + 1) * half_len])
                    xt[c][half] = x_
                    if half == 0:
                        w = pp.tile([CHUNK, 3 * DPC], BF16, tag=f"w{c}")
                        dma_engs[(c + 1) % 3].dma_start(
                            out=w, in_=wqkvT[c * CHUNK:(c + 1) * CHUNK, :])
                        w_sb[c] = w
            wo_sb = []
            for i in range(2):
                w = pp.tile([CHUNK, C], BF16, tag=f"wo{i}")
                nc.sync.dma_start(out=w, in_=woT[i * CHUNK:(i + 1) * CHUNK, :])
                wo_sb.append(w)
            bqk_sb = pp.tile([CHUNK, 4], F32, tag="bqk")
            nc.gpsimd.dma_start(out=bqk_sb, in_=bqk[:, :])
            bv_bc = pp.tile([CHUNK, DPC], F32, tag="bv_bc")
            nc.gpsimd.dma_start(out=bv_bc, in_=bv_row[0:1, :].broadcast_to((CHUNK, DPC)))

            # ---- constants ----
            trimask = pp.tile([CHUNK, CHUNK], BF16, tag="trimask")
            nc.gpsimd.memset(trimask, 1.0)
            # keep 1.0 where t - s >= 0 (upper triangular incl. diagonal)
            nc.gpsimd.affine_select(
                out=trimask, in_=trimask,
                pattern=[[1, CHUNK]], channel_multiplier=-1, base=0,
                compare_op=mybir.AluOpType.is_ge, fill=0.0,
            )

            # ---- persistent activations ----
            # qT/kT per head-pair: [128, T], head (2m+hf) at partitions 64hf..
            qT = [pp.tile([CHUNK, t_len], BF16, tag=f"qT{m}", name=f"qT{m}")
                  for m in range(2)]
            kT = [pp.tile([CHUNK, t_len], BF16, tag=f"kT{m}", name=f"kT{m}")
                  for m in range(2)]
            # v tiles: [s-chunk 128, 4 heads x (64 v | ones)]
            vaug = [pp.tile([CHUNK, HPC * VW], BF16, tag=f"v{j}", name=f"v{j}")
                    for j in range(n_ttile)]
            for j in range(n_ttile):
                ones_cols = vaug[j].rearrange("p (h e) -> p h e", e=VW)[:, :, D:D + 1]
                nc.vector.memset(ones_cols, 1.0)
            aoT = [pp.tile([CHUNK, t_len], BF16, tag=f"aoT{m}", name=f"aoT{m}")
                   for m in range(2)]

            with tc.tile_pool(name="sb2", bufs=1) as sp, \
                 tc.tile_pool(name="dr", bufs=1, space="DRAM") as dr, \
                 tc.tile_pool(name="ps_s", bufs=1, space="PSUM") as pss, \
                 tc.tile_pool(name="ps_q", bufs=1, space="PSUM") as psq, \
                 tc.tile_pool(name="ps_av", bufs=1, space="PSUM") as psav:

                def emit_ph1(st):
                    """Projections for t-strip st: v into vaug, q/k into qT/kT."""
                    s0 = st * STRIP
                    half, off = s0 // half_len, s0 % half_len
                    for tsub in range(STRIP // CHUNK):
                        jt = st * (STRIP // CHUNK) + tsub
                        pv = psq.tile([CHUNK, STRIP], F32, tag="pq", bufs=2,
                                      name=f"pv{jt}")
                        for c in range(n_cchunk):
                            nc.tensor.matmul(
                                pv[:, 0:DPC],
                                xt[c][half][:, off + tsub * CHUNK:
                                            off + (tsub + 1) * CHUNK],
                                w_sb[c][:, 2 * DPC:3 * DPC],
                                start=(c == 0), stop=(c == n_cchunk - 1))
                        nc.vector.tensor_add(
                            vaug[jt].rearrange("p (h e) -> p h e", e=VW)[:, :, 0:D],
                            pv[:, 0:DPC].rearrange("p (h d) -> p h d", d=D),
                            bv_bc.rearrange("p (h d) -> p h d", d=D))
                    for m in range(2):            # head pair
                        for pj in range(2):       # 0 = q, 1 = k
                            pq = psq.tile([CHUNK, STRIP], F32, tag="pq", bufs=2,
                                          name=f"pq{pj}_{m}_{st}")
                            for c in range(n_cchunk):
                                nc.tensor.matmul(
                                    pq,
                                    w_sb[c][:, pj * DPC + m * CHUNK:
                                            pj * DPC + (m + 1) * CHUNK],
                                    xt[c][half][:, off:off + STRIP],
                                    start=(c == 0), stop=(c == n_cchunk - 1))
                            dst = (qT if pj == 0 else kT)[m][:, s0:s0 + STRIP]
                            nc.vector.tensor_scalar_add(
                                dst, pq, bqk_sb[:, 2 * pj + m:2 * pj + m + 1])

                def emit_ph2(st):
                    """Causal attention for t-strip st, both head pairs.

                    Software-pipelined: av(j-1) is emitted AFTER scores(j) so
                    the in-order PE queue overlaps the next score matmul with
                    the ACT exp of the previous chunk."""
                    s0 = st * STRIP
                    nj = 4 * st + 4
                    for pair in range(2):
                        avA = psav.tile([CHUNK, STRIP], F32, tag="avA", bufs=1,
                                        name=f"avA{pair}_{st}")
                        avB = psav.tile([CHUNK, STRIP], F32, tag="avB", bufs=1,
                                        name=f"avB{pair}_{st}")
                        pend = None   # (j, off, L, eAB) awaiting its AV matmuls

                        def emit_av(j, off, L, eAB):
                            for hf, av in ((0, avA), (1, avB)):
                                h = 2 * pair + hf
                                nc.tensor.matmul(
                                    av[0:VW, off:STRIP],
                                    vaug[j][:, h * VW:(h + 1) * VW],
                                    eAB[:, hf * STRIP:hf * STRIP + L],
                                    start=(j == 0), stop=(j == nj - 1),
                                    skip_group_check=True)

                        for j in range(nj):
                            t0 = max(s0, CHUNK * j)
                            L = s0 + STRIP - t0
                            off = t0 - s0
                            sAB = pss.tile([CHUNK, 2 * STRIP], F32, tag="sAB",
                                           bufs=2, name=f"s{pair}_{st}_{j}")
                            for hf in range(2):
                                nc.tensor.matmul(
                                    sAB[:, hf * STRIP:hf * STRIP + L],
                                    kT[pair][64 * hf:64 * hf + 64,
                                             j * CHUNK:(j + 1) * CHUNK],
                                    qT[pair][64 * hf:64 * hf + 64, t0:t0 + L],
                                    start=True, stop=True)
                            if pend is not None:
                                emit_av(*pend)
                            eAB = sp.tile([CHUNK, 2 * STRIP], BF16, tag="eAB",
                                          bufs=3, name=f"e{pair}_{st}_{j}")
                            if L == STRIP:
                                nc.scalar.activation(eAB, sAB, Exp)
                            else:
                                nc.scalar.activation(
                                    eAB.rearrange("p (h f) -> p h f",
                                                  f=STRIP)[:, :, 0:L],
                                    sAB.rearrange("p (h f) -> p h f",
                                                  f=STRIP)[:, :, 0:L],
                                    Exp)
                            if CHUNK * j >= s0:  # diagonal tile
                                nc.vector.tensor_mul(
                                    eAB[:, 0:CHUNK], eAB[:, 0:CHUNK], trimask)
                                nc.vector.tensor_mul(
                                    eAB[:, STRIP:STRIP + CHUNK],
                                    eAB[:, STRIP:STRIP + CHUNK], trimask)
                            pend = (j, off, L, eAB)
                        emit_av(*pend)
                        # normalize: denom rows -> DRAM-reshaped [128,8] so the
                        # reciprocal uses all 128 DVE lanes, then broadcast back
                        bcf = sp.tile([CHUNK, STRIP], BF16, tag="bcf", bufs=2,
                                      name=f"bcf{pair}_{st}")
                        dnd = dr.tile([2, STRIP], F32, tag="dnd", bufs=2,
                                      name=f"dnd_{pair}_{st}")
                        for hf, av in ((0, avA), (1, avB)):
                            drow = sp.tile([1, STRIP], F32, tag=f"dr{hf}",
                                           bufs=2, name=f"dr{hf}_{pair}_{st}")
                            nc.vector.tensor_copy(drow, av[D:D + 1, :])
                            nc.gpsimd.dma_start(out=dnd[hf:hf + 1, :], in_=drow)
                        den128 = sp.tile([CHUNK, 8], F32, tag="dn128", bufs=2,
                                         name=f"dn128_{pair}_{st}")
                        nc.gpsimd.dma_start(
                            out=den128,
                            in_=dnd.rearrange("a b -> (a b)").rearrange(
                                "(p f) -> p f", p=CHUNK))
                        rec128 = sp.tile([CHUNK, 8], BF16, tag="rc128", bufs=2,
                                         name=f"rc128_{pair}_{st}")
                        with nc.allow_low_precision(reason="1/denom in bf16"):
                            nc.vector.reciprocal(rec128, den128)
                        recd = dr.tile([2, STRIP], BF16, tag="recd", bufs=2,
                                       name=f"recd_{pair}_{st}")
                        nc.gpsimd.dma_start(
                            out=recd.rearrange("a b -> (a b)").rearrange(
                                "(p f) -> p f", p=CHUNK),
                            in_=rec128)
                        for hf in range(2):
                            nc.gpsimd.dma_start(
                                out=bcf[64 * hf:64 * hf + D, :],
                                in_=recd[hf:hf + 1, :].broadcast_to((D, STRIP)))
                        # head A: partitions align -> fused multiply
                        nc.vector.tensor_mul(
                            aoT[pair][0:D, s0:s0 + STRIP],
                            avA[0:D, :], bcf[0:D, :])
                        # head B: cross-partition fused multiply (dst base 64,
                        # av src base 0 — shifted copies are proven to work)
                        nc.vector.tensor_mul(
                            aoT[pair][64:64 + D, s0:s0 + STRIP],
                            avB[0:D, :], bcf[64:64 + D, :])

                def emit_ph3(st):
                    """Out-proj partial for t-strip st; y stored bf16."""
                    for tsub in range(STRIP // CHUNK):
                        jt = st * (STRIP // CHUNK) + tsub
                        ysb = sp.tile([CHUNK, C], BF16, tag="ysb", bufs=3,
                                      name=f"ysb{jt}")
                        for js in range(2):
                            py = psq.tile([CHUNK, STRIP], F32, tag="pq", bufs=2,
                                          name=f"py{jt}_{js}")
                            for pair in range(2):
                                nc.tensor.matmul(
                                    py,
                                    aoT[pair][:, jt * CHUNK:(jt + 1) * CHUNK],
                                    wo_sb[pair][:, js * STRIP:(js + 1) * STRIP],
                                    start=(pair == 0), stop=(pair == 1))
                            nc.vector.tensor_copy(
                                ysb[:, js * STRIP:(js + 1) * STRIP], py)
                        nc.gpsimd.dma_start(
                            out=y[jt * CHUNK:(jt + 1) * CHUNK, :], in_=ysb)

                for st in range(n_strip):
                    emit_ph1(st)
                    if st > 0:
                        emit_ph3(st - 1)
                    emit_ph2(st)
                emit_ph3(n_strip - 1)

    if fix_waits:
        split_excess_waits(nc)
    return nc


def make_in_maps(x, wq, bq, wk, bk, wv, bv, wo, bo=None, t_len=T):
    """Build the 8 per-core input dicts from full inputs (bo handled on host)."""
    in_maps = []
    scale = 1.0 / np.sqrt(np.float32(D))
    for core in range(NCORES):
        b, hg = core // 4, core % 4
        sl = slice(DPC * hg, DPC * (hg + 1))
        wqs = (wq[sl] * scale).astype(np.float32)
        bqs = (bq[sl] * scale).astype(np.float32)
        wqkvT = np.concatenate([wqs.T, wk[sl].T, wv[sl].T], axis=1)
        bqk = np.stack([bqs[0:CHUNK], bqs[CHUNK:2 * CHUNK],
                        bk[sl][0:CHUNK], bk[sl][CHUNK:2 * CHUNK]], axis=1)
        in_maps.append({
            "xT": np.ascontiguousarray(x[b, :t_len].T).astype(BF16_NP),
            "wqkvT": np.ascontiguousarray(wqkvT).astype(BF16_NP),
            "woT": np.ascontiguousarray(wo[:, sl].T).astype(BF16_NP),
            "bqk": np.ascontiguousarray(bqk, dtype=np.float32),
            "bv_row": np.ascontiguousarray(bv[sl][None, :], dtype=np.float32),
        })
    return in_maps


def gather_output(results, bo, t_len=T):
    ys = [np.asarray(results[i]["y"], dtype=np.float32) for i in range(NCORES)]
    out = np.stack([ys[0] + ys[1] + ys[2] + ys[3],
                    ys[4] + ys[5] + ys[6] + ys[7]])
    return (out + np.asarray(bo, dtype=np.float32)[None, None, :]).astype(np.float32)


_NC_CACHE = {}


def _get_nc(t_len=T):
    if t_len not in _NC_CACHE:
        _NC_CACHE[t_len] = build_nc(t_len)
    return _NC_CACHE[t_len]


def kernel(x, wq, bq, wk, bk, wv, bv, wo, bo, mask=None, **_unused):
    """Full-input entry point: shard, run on 8 NeuronCores, gather."""
    from concourse.bass_utils import run_bass_kernel_spmd

    x = np.asarray(x, dtype=np.float32)
    in_maps = make_in_maps(x, np.asarray(wq, np.float32), np.asarray(bq, np.float32),
                           np.asarray(wk, np.float32), np.asarray(bk, np.float32),
                           np.asarray(wv, np.float32), np.asarray(bv, np.float32),
                           np.asarray(wo, np.float32))
    nc = _get_nc(T)
    res = run_bass_kernel_spmd(nc, in_maps, list(range(NCORES)))
    return gather_output(res.results, bo)


# revision 16
# speedup vs baseline: 1.3926x; 1.2558x over previous
"""Trainium2 Bass kernel: causal multi-head attention block (B=2, T=2048, C=1024, H=16).

Sharding: 8 cores = 2 (batch) x 4 (head groups of 4 heads).  Each core computes
q/k/v projections for its 4 heads, causal attention, and a partial out-proj
(rows of wo for its head slice).  Host sums the 4 partials per batch element
and adds bo.

v2 restructure (vs baseline):
  - Strip-pipelined: for each 512-row strip st, emit out-proj(st-1),
    projections(st), attention(st).  Causality means attention strip st only
    needs k/v prefix data, so ACT-exp work starts ~12us in instead of ~45us,
    and the PE never idles long enough for HAM to re-throttle to 1.2 GHz.
  - q/k stored per head-pair [128, T] with head h at partitions 64*(h%2)..;
    score matmuls contract 64 partitions at matching base partition.  No
    zero-padding, no big startup memsets.
  - exp batched 2 heads per ACT instruction (sAB [128, 1024] across 2 PSUM
    banks) -> ~80 instead of 160 activations.
  - Softmax denominator via ones-column in v (PSUM row 64); reciprocal +
    SBUF->SBUF broadcast DMA + fused normalize, all on-chip (no DRAM bounce).
  - y stored bf16 (halves DMA-out), bias bo + partial sum on host.
  - Scalar engine does ONLY exp; DMA triggers live on sync/gpsimd.
"""

import os
import sys

import numpy as np

try:
    import ml_dtypes
    BF16_NP = ml_dtypes.bfloat16
except ImportError:  # pragma: no cover
    BF16_NP = None

for _p in ("/opt/trn_rl_repo", "/root/.axon_site/_ro/trn_rl_repo"):
    if os.path.isdir(_p) and _p not in sys.path:
        sys.path.append(_p)

import concourse.bass as bass  # noqa: E402
import concourse.mybir as mybir  # noqa: E402
import concourse.tile as tile  # noqa: E402

F32 = mybir.dt.float32
BF16 = mybir.dt.bfloat16

B, T, C, H = 2, 2048, 1024, 16
D = C // H          # 64
HPC = 4             # heads per core
DPC = HPC * D       # 256 head-dims per core
NCORES = 8

CHUNK = 128         # contraction / s-chunk granularity
STRIP = 512         # t-strip width
VW = D + 1          # 65: v columns + ones column per head

_CTRL_TYPES = (mybir.InstDrain, mybir.InstNoOp, mybir.InstEventSemaphore)


def split_excess_waits(nc, lim=1):
    """Walrus in this toolchain accepts at most one sync-wait per instruction.
    Move extra waits onto same-engine NoOps inserted just before the owner."""
    k = 0
    for fn in nc.m.functions:
        for blk in fn.blocks:
            out = []
            changed = False
            for inst in blk.instructions:
                si = inst.sync_info
                if si is not None and si.on_wait and len(si.on_wait) > lim:
                    waits = list(si.on_wait)
                    extra, keep = waits[:-lim], waits[-lim:]
                    for w in extra:
                        nop = mybir.InstNoOp(name=f"waitfix_{k}", ins=[], outs=[])
                        k += 1
                        nop.engine = inst.engine
                        nop.sync_info = mybir.SyncInfo(on_wait=[w], on_update=[])
                        out.append(nop)
                    si.on_wait = keep
                    changed = True
                out.append(inst)
            if changed:
                blk.instructions = out
    return k


def build_nc(t_len=T, fix_waits=True):
    """Build the per-core SPMD Bass program (same program on all 8 cores)."""
    assert t_len % STRIP == 0
    n_strip = t_len // STRIP          # 4 at full size
    n_ttile = t_len // CHUNK          # 16 at full size
    n_cchunk = C // CHUNK             # 8
    half_len = min(t_len, 1024)

    nc = bass.Bass(target_bir_lowering=False)

    xT = nc.dram_tensor("xT", [C, t_len], BF16, kind="ExternalInput")
    wqkvT = nc.dram_tensor("wqkvT", [C, 3 * DPC], BF16, kind="ExternalInput")
    woT = nc.dram_tensor("woT", [DPC, C], BF16, kind="ExternalInput")
    bqk = nc.dram_tensor("bqk", [CHUNK, 4], F32, kind="ExternalInput")
    bv_row = nc.dram_tensor("bv_row", [1, DPC], F32, kind="ExternalInput")
    y = nc.dram_tensor("y", [t_len, C], BF16, kind="ExternalOutput")

    Exp = mybir.ActivationFunctionType.Exp

    with tile.TileContext(nc) as tc:
        with tc.tile_pool(name="persist", bufs=1) as pp:
            # ---- input DMAs first, spread over 4 engine queues so strip-0
            # data lands fast and the PE can start ASAP ----
            dma_engs = [nc.sync, nc.gpsimd, nc.scalar]
            n_half = t_len // half_len
            xt = [[None] * n_half for _ in range(n_cchunk)]
            w_sb = [None] * n_cchunk
            for half in range(n_half):
                for c in range(n_cchunk):
                    x_ = pp.tile([CHUNK, half_len], BF16, tag=f"x{c}_{half}",
                                 name=f"x{c}_{half}")
                    dma_engs[c % 3].dma_start(
                        out=x_,
                        in_=xT[c * CHUNK:(c + 1) * CHUNK,
                               half * half_len:(half + 1) * half_len])
                    xt[c][half] = x_
                    if half == 0:
                        w = pp.tile([CHUNK, 3 * DPC], BF16, tag=f"w{c}")
                        dma_engs[(c + 1) % 3].dma_start(
                            out=w, in_=wqkvT[c * CHUNK:(c + 1) * CHUNK, :])
                        w_sb[c] = w
            wo_sb = []
            for i in range(2):
                w = pp.tile([CHUNK, C], BF16, tag=f"wo{i}")
                nc.sync.dma_start(out=w, in_=woT[i * CHUNK:(i + 1) * CHUNK, :])
                wo_sb.append(w)
            bqk_sb = pp.tile([CHUNK, 4], F32, tag="bqk")
            nc.gpsimd.dma_start(out=bqk_sb, in_=bqk[:, :])
            bv_bc = pp.tile([CHUNK, DPC], F32, tag="bv_bc")
            nc.gpsimd.dma_start(out=bv_bc, in_=bv_row[0:1, :].broadcast_to((CHUNK, DPC)))

            # ---- constants ----
            trimask = pp.tile([CHUNK, CHUNK], BF16, tag="trimask")
            nc.gpsimd.memset(trimask, 1.0)
            # keep 1.0 where t - s >= 0 (upper triangular incl. diagonal)
            nc.gpsimd.affine_select(
                out=trimask, in_=trimask,
                pattern=[[1, CHUNK]], channel_multiplier=-1, base=0,
                compare_op=mybir.AluOpType.is_ge, fill=0.0,
            )

            # ---- persistent activations ----
            # qT/kT per head-pair: [128, T], head (2m+hf) at partitions 64hf..
            qT = [pp.tile([CHUNK, t_len], BF16, tag=f"qT{m}", name=f"qT{m}")
                  for m in range(2)]
            kT = [pp.tile([CHUNK, t_len], BF16, tag=f"kT{m}", name=f"kT{m}")
                  for m in range(2)]
            # v tiles: [s-chunk 128, 4 heads x (64 v | ones)]
            vaug = [pp.tile([CHUNK, HPC * VW], BF16, tag=f"v{j}", name=f"v{j}")
                    for j in range(n_ttile)]
            for j in range(n_ttile):
                ones_cols = vaug[j].rearrange("p (h e) -> p h e", e=VW)[:, :, D:D + 1]
                nc.vector.memset(ones_cols, 1.0)
            aoT = [pp.tile([CHUNK, t_len], BF16, tag=f"aoT{m}", name=f"aoT{m}")
                   for m in range(2)]

            with tc.tile_pool(name="sb2", bufs=1) as sp, \
                 tc.tile_pool(name="dr", bufs=1, space="DRAM") as dr, \
                 tc.tile_pool(name="ps_s", bufs=1, space="PSUM") as pss, \
                 tc.tile_pool(name="ps_q", bufs=1, space="PSUM") as psq, \
                 tc.tile_pool(name="ps_av", bufs=1, space="PSUM") as psav:

                def emit_ph1(st):
                    """Projections for t-strip st: v into vaug, q/k into qT/kT."""
                    s0 = st * STRIP
                    half, off = s0 // half_len, s0 % half_len
                    for tsub in range(STRIP // CHUNK):
                        jt = st * (STRIP // CHUNK) + tsub
                        pv = psq.tile([CHUNK, STRIP], F32, tag="pq", bufs=2,
                                      name=f"pv{jt}")
                        for c in range(n_cchunk):
                            nc.tensor.matmul(
                                pv[:, 0:DPC],
                                xt[c][half][:, off + tsub * CHUNK:
                                            off + (tsub + 1) * CHUNK],
                                w_sb[c][:, 2 * DPC:3 * DPC],
                                start=(c == 0), stop=(c == n_cchunk - 1))
                        nc.vector.tensor_add(
                            vaug[jt].rearrange("p (h e) -> p h e", e=VW)[:, :, 0:D],
                            pv[:, 0:DPC].rearrange("p (h d) -> p h d", d=D),
                            bv_bc.rearrange("p (h d) -> p h d", d=D))
                    for m in range(2):            # head pair
                        for pj in range(2):       # 0 = q, 1 = k
                            pq = psq.tile([CHUNK, STRIP], F32, tag="pq", bufs=2,
                                          name=f"pq{pj}_{m}_{st}")
                            for c in range(n_cchunk):
                                nc.tensor.matmul(
                                    pq,
                                    w_sb[c][:, pj * DPC + m * CHUNK:
                                            pj * DPC + (m + 1) * CHUNK],
                                    xt[c][half][:, off:off + STRIP],
                                    start=(c == 0), stop=(c == n_cchunk - 1))
                            dst = (qT if pj == 0 else kT)[m][:, s0:s0 + STRIP]
                            nc.vector.tensor_scalar_add(
                                dst, pq, bqk_sb[:, 2 * pj + m:2 * pj + m + 1])

                def emit_ph2(st):
                    """Causal attention for t-strip st, both head pairs.

                    Per pair: ALL score matmuls + exps first (deep eAB ring),
                    then all AV matmuls.  The score stream of pair p+1 keeps
                    the PE busy while pair p's normalize chain (which gates
                    the single-buffered av banks) completes."""
                    s0 = st * STRIP
                    nj = 4 * st + 4
                    for pair in range(2):
                        es = []
                        for j in range(nj):
                            t0 = max(s0, CHUNK * j)
                            L = s0 + STRIP - t0
                            sAB = pss.tile([CHUNK, 2 * STRIP], F32, tag="sAB",
                                           bufs=2, name=f"s{pair}_{st}_{j}")
                            for hf in range(2):
                                nc.tensor.matmul(
                                    sAB[:, hf * STRIP:hf * STRIP + L],
                                    kT[pair][64 * hf:64 * hf + 64,
                                             j * CHUNK:(j + 1) * CHUNK],
                                    qT[pair][64 * hf:64 * hf + 64, t0:t0 + L],
                                    start=True, stop=True)
                            eAB = sp.tile([CHUNK, 2 * STRIP], BF16, tag="eAB",
                                          bufs=18, name=f"e{pair}_{st}_{j}")
                            if L == STRIP:
                                nc.scalar.activation(eAB, sAB, Exp)
                            else:
                                nc.scalar.activation(
                                    eAB.rearrange("p (h f) -> p h f",
                                                  f=STRIP)[:, :, 0:L],
                                    sAB.rearrange("p (h f) -> p h f",
                                                  f=STRIP)[:, :, 0:L],
                                    Exp)
                            if CHUNK * j >= s0:  # diagonal tile
                                nc.vector.tensor_mul(
                                    eAB[:, 0:CHUNK], eAB[:, 0:CHUNK], trimask)
                                nc.vector.tensor_mul(
                                    eAB[:, STRIP:STRIP + CHUNK],
                                    eAB[:, STRIP:STRIP + CHUNK], trimask)
                            es.append((t0 - s0, L, eAB))
                        avA = psav.tile([CHUNK, STRIP], F32, tag="avA", bufs=1,
                                        name=f"avA{pair}_{st}")
                        avB = psav.tile([CHUNK, STRIP], F32, tag="avB", bufs=1,
                                        name=f"avB{pair}_{st}")
                        for j, (off, L, eAB) in enumerate(es):
                            for hf, av in ((0, avA), (1, avB)):
                                h = 2 * pair + hf
                                nc.tensor.matmul(
                                    av[0:VW, off:STRIP],
                                    vaug[j][:, h * VW:(h + 1) * VW],
                                    eAB[:, hf * STRIP:hf * STRIP + L],
                                    start=(j == 0), stop=(j == nj - 1),
                                    skip_group_check=True)
                        # normalize: denom rows -> DRAM-reshaped [128,8] so the
                        # reciprocal uses all 128 DVE lanes, then broadcast back
                        bcf = sp.tile([CHUNK, STRIP], BF16, tag="bcf", bufs=2,
                                      name=f"bcf{pair}_{st}")
                        drow2 = sp.tile([1, 2 * STRIP], F32, tag="drow2",
                                        bufs=2, name=f"drow2_{pair}_{st}")
                        for hf, av in ((0, avA), (1, avB)):
                            nc.vector.tensor_copy(
                                drow2[:, hf * STRIP:(hf + 1) * STRIP],
                                av[D:D + 1, :])
                        dnd = dr.tile([2, STRIP], F32, tag="dnd", bufs=2,
                                      name=f"dnd_{pair}_{st}")
                        nc.gpsimd.dma_start(
                            out=dnd.rearrange("a b -> (a b)")[None, :], in_=drow2)
                        den128 = sp.tile([CHUNK, 8], F32, tag="dn128", bufs=2,
                                         name=f"dn128_{pair}_{st}")
                        nc.gpsimd.dma_start(
                            out=den128,
                            in_=dnd.rearrange("a b -> (a b)").rearrange(
                                "(p f) -> p f", p=CHUNK))
                        rec128 = sp.tile([CHUNK, 8], BF16, tag="rc128", bufs=2,
                                         name=f"rc128_{pair}_{st}")
                        with nc.allow_low_precision(reason="1/denom in bf16"):
                            nc.vector.reciprocal(rec128, den128)
                        recd = dr.tile([2, STRIP], BF16, tag="recd", bufs=2,
                                       name=f"recd_{pair}_{st}")
                        nc.gpsimd.dma_start(
                            out=recd.rearrange("a b -> (a b)").rearrange(
                                "(p f) -> p f", p=CHUNK),
                            in_=rec128)
                        for hf in range(2):
                            nc.gpsimd.dma_start(
                                out=bcf[64 * hf:64 * hf + D, :],
                                in_=recd[hf:hf + 1, :].broadcast_to((D, STRIP)))
                        # head A: partitions align -> fused multiply
                        nc.vector.tensor_mul(
                            aoT[pair][0:D, s0:s0 + STRIP],
                            avA[0:D, :], bcf[0:D, :])
                        # head B: cross-partition fused multiply (dst base 64,
                        # av src base 0 — shifted copies are proven to work)
                        nc.vector.tensor_mul(
                            aoT[pair][64:64 + D, s0:s0 + STRIP],
                            avB[0:D, :], bcf[64:64 + D, :])

                def emit_ph3(st):
                    """Out-proj partial for t-strip st; y stored bf16."""
                    for tsub in range(STRIP // CHUNK):
                        jt = st * (STRIP // CHUNK) + tsub
                        ysb = sp.tile([CHUNK, C], BF16, tag="ysb", bufs=3,
                                      name=f"ysb{jt}")
                        for js in range(2):
                            py = psq.tile([CHUNK, STRIP], F32, tag="pq", bufs=2,
                                          name=f"py{jt}_{js}")
                            for pair in range(2):
                                nc.tensor.matmul(
                                    py,
                                    aoT[pair][:, jt * CHUNK:(jt + 1) * CHUNK],
                                    wo_sb[pair][:, js * STRIP:(js + 1) * STRIP],
                                    start=(pair == 0), stop=(pair == 1))
                            nc.vector.tensor_copy(
                                ysb[:, js * STRIP:(js + 1) * STRIP], py)
                        nc.gpsimd.dma_start(
                            out=y[jt * CHUNK:(jt + 1) * CHUNK, :], in_=ysb)

                for st in range(n_strip):
                    emit_ph1(st)
                    if st > 0:
                        emit_ph3(st - 1)
                    emit_ph2(st)
                emit_ph3(n_strip - 1)

    if fix_waits:
        split_excess_waits(nc)
    return nc


def make_in_maps(x, wq, bq, wk, bk, wv, bv, wo, bo=None, t_len=T):
    """Build the 8 per-core input dicts from full inputs (bo handled on host)."""
    in_maps = []
    scale = 1.0 / np.sqrt(np.float32(D))
    for core in range(NCORES):
        b, hg = core // 4, core % 4
        sl = slice(DPC * hg, DPC * (hg + 1))
        wqs = (wq[sl] * scale).astype(np.float32)
        bqs = (bq[sl] * scale).astype(np.float32)
        wqkvT = np.concatenate([wqs.T, wk[sl].T, wv[sl].T], axis=1)
        bqk = np.stack([bqs[0:CHUNK], bqs[CHUNK:2 * CHUNK],
                        bk[sl][0:CHUNK], bk[sl][CHUNK:2 * CHUNK]], axis=1)
        in_maps.append({
            "xT": np.ascontiguousarray(x[b, :t_len].T).astype(BF16_NP),
            "wqkvT": np.ascontiguousarray(wqkvT).astype(BF16_NP),
            "woT": np.ascontiguousarray(wo[:, sl].T).astype(BF16_NP),
            "bqk": np.ascontiguousarray(bqk, dtype=np.float32),
            "bv_row": np.ascontiguousarray(bv[sl][None, :], dtype=np.float32),
        })
    return in_maps


def gather_output(results, bo, t_len=T):
    ys = [np.asarray(results[i]["y"], dtype=np.float32) for i in range(NCORES)]
    out = np.stack([ys[0] + ys[1] + ys[2] + ys[3],
                    ys[4] + ys[5] + ys[6] + ys[7]])
    return (out + np.asarray(bo, dtype=np.float32)[None, None, :]).astype(np.float32)


_NC_CACHE = {}


def _get_nc(t_len=T):
    if t_len not in _NC_CACHE:
        _NC_CACHE[t_len] = build_nc(t_len)
    return _NC_CACHE[t_len]


def kernel(x, wq, bq, wk, bk, wv, bv, wo, bo, mask=None, **_unused):
    """Full-input entry point: shard, run on 8 NeuronCores, gather."""
    from concourse.bass_utils import run_bass_kernel_spmd

    x = np.asarray(x, dtype=np.float32)
    in_maps = make_in_maps(x, np.asarray(wq, np.float32), np.asarray(bq, np.float32),
                           np.asarray(wk, np.float32), np.asarray(bk, np.float32),
                           np.asarray(wv, np.float32), np.asarray(bv, np.float32),
                           np.asarray(wo, np.float32))
    nc = _get_nc(T)
    res = run_bass_kernel_spmd(nc, in_maps, list(range(NCORES)))
    return gather_output(res.results, bo)
